# revision 9
# baseline (speedup 1.0000x reference)
"""BilinearAttention Trainium2 kernel (8 NeuronCores, SPMD, no collectives).

Problem (hardcoded): B=4, C=256, H=W=64 (HW=4096)
  theta = convbn_theta(x)   -> [B, 32, HW]
  phi   = convbn_phi(fea)   -> [B, 32, HW]
  g     = convbn_g(fea)     -> [B, 128, HW]
  attn  = softmax_m(theta^T . phi)          [B, HW(n), HW(m)]
  out   = g @ attn^T                         [B, 128, HW]
  final = convbn_fin(out)                    [B, 256, H, W]

Sharding: core k handles (b = k//2, n-half h = k%2): 2048 queries x 4096 keys.

Device algorithm per core (matmuls in f32r ~ tf32 by default, psum fp32):
  - BN folded into conv weights on host; g bias folded into fin bias
    (attn rows sum to 1); theta/phi biases applied on-chip.
  - theta_rep [128,2048]: 4 partition-group copies of theta[32, n] for
    row-tiled K=32 QK matmuls. phi_rep [128, 4096] likewise.
  - gT [128, 4096]: column block mt holds g^T of m-tile mt ([128 m, 128 c]).
  - 128 tasks (nb-major; task = (m-tile mt, n-block nb)), grouped in triples:
      QK:  logitsT[m,n] psum = phi_mt^T theta_nb  (row-packed tile_position)
      exp: ACT psum->sbuf (one [128, 3*512] instr per triple; no max
           subtraction: |logit| < ~40 so exp fits fp32 comfortably)
      AV:  av_ps[nb] += gT_mt^T . pT     (accumulate over 32 m-tiles)
      s:   s_ps[nb]  += ones^T . pT      (softmax denominators)
  - Tail: r = 1/s; broadcast via (1/32)-matmul; avn = av*r; fin conv; + bias.
"""
import os
import numpy as np
from contextlib import ExitStack

B, C, HW = 4, 256, 4096
NSH = HW // 2           # 2048 queries per core
NCORES = 8
BN_EPS = 1e-5
NB = 4                  # n-blocks of 512 per core
MT = 32                 # m-tiles of 128
TRIPLE = 3              # QK tasks per exp instruction

MM_DTYPE = os.environ.get("K_MM_DTYPE", "mixed")   # f32r | bf16 | mixed
QK_PACK = os.environ.get("K_QK_PACK", "1") == "1"

_CACHE = {}


def _np_dtypes():
    import ml_dtypes
    bf = ml_dtypes.bfloat16
    if MM_DTYPE == "bf16":
        return bf, bf
    if MM_DTYPE == "f32r":
        return np.float32, np.float32
    return np.float32, bf  # mixed: (DQ, DA)


def _build(trace_sim=False, repeat=1):
    key = ("nc", repeat, MM_DTYPE, QK_PACK)
    if key in _CACHE:
        return _CACHE[key]
    import concourse.bacc as bacc
    import concourse.tile as tile
    from concourse import mybir

    F32 = mybir.dt.float32
    # DQ: logits path (theta/phi/QK). DA: attention-weight path (pt/gT/ones/AV/s).
    DQ = mybir.dt.bfloat16 if MM_DTYPE == "bf16" else mybir.dt.float32r
    DA = mybir.dt.float32r if MM_DTYPE == "f32r" else mybir.dt.bfloat16
    AF = mybir.ActivationFunctionType

    nc = bacc.Bacc("TRN2", target_bir_lowering=False, debug=False,
                   num_devices=NCORES)

    xk_d = nc.dram_tensor("xk", [C, NSH], DQ, kind="ExternalInput").ap()
    fea_d = nc.dram_tensor("fea", [C, HW], DQ, kind="ExternalInput").ap()
    feab_d = (nc.dram_tensor("feab", [C, HW], DA, kind="ExternalInput").ap()
              if MM_DTYPE == "mixed" else fea_d)
    thw_d = nc.dram_tensor("thw", [C, 128], DQ, kind="ExternalInput").ap()
    phw_d = nc.dram_tensor("phw", [C, 128], DQ, kind="ExternalInput").ap()
    gwt_d = nc.dram_tensor("gwt", [C, 128], DA, kind="ExternalInput").ap()
    fwt_d = nc.dram_tensor("fwt", [128, 256], DQ, kind="ExternalInput").ap()
    ones_d = nc.dram_tensor("ones", [128, 128], DA, kind="ExternalInput").ap()
    inv32_d = nc.dram_tensor("inv32", [32, 128], DQ, kind="ExternalInput").ap()
    tth_d = nc.dram_tensor("tth", [128, 1], F32, kind="ExternalInput").ap()
    tph_d = nc.dram_tensor("tph", [128, 1], F32, kind="ExternalInput").ap()
    tfn_d = nc.dram_tensor("tfn", [128, 2], F32, kind="ExternalInput").ap()
    out_d = nc.dram_tensor("out", [256, NSH], F32, kind="ExternalOutput").ap()

    with tile.TileContext(nc, trace_sim=trace_sim) as tc, ExitStack() as ctx:
      def body():
        consts = ctx.enter_context(tc.tile_pool(name="consts", bufs=1))
        inbufs = ctx.enter_context(tc.tile_pool(name="inbufs", bufs=4))
        big = ctx.enter_context(tc.tile_pool(name="big", bufs=1))
        ptp = ctx.enter_context(tc.tile_pool(name="ptp", bufs=2))
        psum = ctx.enter_context(tc.tile_pool(name="psum", bufs=2, space="PSUM"))

        def dmac(name, shape, dt, src):
            t = consts.tile(shape, dt, name=name, tag=name)
            nc.sync.dma_start(out=t, in_=src)
            return t

        thw = [dmac(f"thw{i}", [128, 128], DQ, thw_d[128 * i:128 * (i + 1), :]) for i in range(2)]
        phw = [dmac(f"phw{i}", [128, 128], DQ, phw_d[128 * i:128 * (i + 1), :]) for i in range(2)]
        gwt = [dmac(f"gwt{i}", [128, 128], DA, gwt_d[128 * i:128 * (i + 1), :]) for i in range(2)]
        fwt = dmac("fwt_t", [128, 256], DQ, fwt_d)
        ones = dmac("ones_t", [128, 128], DA, ones_d)
        inv32 = dmac("inv32_t", [32, 128], DQ, inv32_d)
        tth = dmac("tth_t", [128, 1], F32, tth_d)
        tph = dmac("tph_t", [128, 1], F32, tph_d)
        tfn = dmac("tfn_t", [128, 2], F32, tfn_d)

        theta_rep = big.tile([128, NSH], DQ)
        phi_rep = big.tile([128, HW], DQ)
        gT = big.tile([128, HW], DA)
        av_sb = big.tile([128, NSH], F32)
        avn = big.tile([128, NSH], DQ)
        s_sb = big.tile([32, NSH], DQ)
        r_sb = big.tile([32, NSH], DQ)
        out_sb = big.tile([128, 2 * NSH], F32)

        # ---- prologue: theta conv over the core's n-range ----
        for j in range(4):
            xk0 = inbufs.tile([128, 512], DQ, tag="xk0")
            xk1 = inbufs.tile([128, 512], DQ, tag="xk1")
            nc.sync.dma_start(out=xk0, in_=xk_d[0:128, 512 * j:512 * (j + 1)])
            nc.sync.dma_start(out=xk1, in_=xk_d[128:256, 512 * j:512 * (j + 1)])
            ps = psum.tile([128, 512], F32, tag="qk")
            nc.tensor.matmul(ps, lhsT=thw[0], rhs=xk0, start=True, stop=False)
            nc.tensor.matmul(ps, lhsT=thw[1], rhs=xk1, start=False, stop=True)
            nc.vector.tensor_scalar_add(theta_rep[:, 512 * j:512 * (j + 1)], ps, tth)

        # ---- prologue: phi conv + gT conv per 512-wide m-chunk ----
        for j in range(8):
            f0 = inbufs.tile([128, 512], DQ, tag="f0")
            f1 = inbufs.tile([128, 512], DQ, tag="f1")
            nc.sync.dma_start(out=f0, in_=fea_d[0:128, 512 * j:512 * (j + 1)])
            nc.sync.dma_start(out=f1, in_=fea_d[128:256, 512 * j:512 * (j + 1)])
            ps = psum.tile([128, 512], F32, tag="qk")
            nc.tensor.matmul(ps, lhsT=phw[0], rhs=f0, start=True, stop=False)
            nc.tensor.matmul(ps, lhsT=phw[1], rhs=f1, start=False, stop=True)
            nc.vector.tensor_scalar_add(phi_rep[:, 512 * j:512 * (j + 1)], ps, tph)
            if MM_DTYPE == "mixed":
                f0b = inbufs.tile([128, 512], DA, tag="f0b")
                f1b = inbufs.tile([128, 512], DA, tag="f1b")
                nc.sync.dma_start(out=f0b, in_=feab_d[0:128, 512 * j:512 * (j + 1)])
                nc.sync.dma_start(out=f1b, in_=feab_d[128:256, 512 * j:512 * (j + 1)])
            else:
                f0b, f1b = f0, f1
            ps2 = psum.tile([128, 512], F32, tag="qk")
            for t in range(4):
                sl = slice(128 * t, 128 * (t + 1))
                nc.tensor.matmul(ps2[:, sl], lhsT=f0b[:, sl], rhs=gwt[0],
                                 start=True, stop=False)
                nc.tensor.matmul(ps2[:, sl], lhsT=f1b[:, sl], rhs=gwt[1],
                                 start=False, stop=True)
            nc.vector.tensor_copy(gT[:, 512 * j:512 * (j + 1)], ps2)

        # ---- main loop ----
        tasks = [(i % MT, i // MT) for i in range(MT * NB)]  # (mt, nb), nb-major
        triples = [tasks[i:i + TRIPLE] for i in range(0, len(tasks), TRIPLE)]
        nt = len(triples)
        av_ps = [None] * NB
        s_ps = [None] * NB
        quads = [None] * nt
        pts = [None] * nt

        def emit_qk(i):
            grp = triples[i]
            q = psum.tile([128, 512 * len(grp)], F32, tag="qk", name=f"q{i}")
            quads[i] = q
            for jj, (mt, nb) in enumerate(grp):
                r = (mt % 4) if QK_PACK else 0
                kw = dict(tile_position=(32 * r, 0)) if QK_PACK else {}
                nc.tensor.matmul(
                    q[:, 512 * jj:512 * (jj + 1)],
                    lhsT=phi_rep[32 * r:32 * (r + 1), 128 * mt:128 * (mt + 1)],
                    rhs=theta_rep[32 * r:32 * (r + 1), 512 * nb:512 * (nb + 1)],
                    start=True, stop=True, **kw,
                )

        def emit_exp(i):
            q = quads[i]
            pt = ptp.tile([128, q.shape[-1]], DA, tag="pt", name=f"pt{i}")
            pts[i] = pt
            nc.scalar.activation(out=pt, in_=q, func=AF.Exp)

        def emit_avs(i):
            grp = triples[i]
            pt = pts[i]
            for jj, (mt, nb) in enumerate(grp):
                if mt == 0:
                    av_ps[nb] = psum.tile([128, 512], F32, tag="av", bufs=1,
                                          name=f"av_ps{nb}")
                    s_ps[nb] = psum.tile([128, 512], F32, tag="sp", bufs=1,
                                         name=f"s_ps{nb}")
                sl = slice(512 * jj, 512 * (jj + 1))
                nc.tensor.matmul(av_ps[nb], lhsT=gT[:, 128 * mt:128 * (mt + 1)],
                                 rhs=pt[:, sl], start=(mt == 0), stop=(mt == MT - 1),
                                 skip_group_check=True)
                nc.tensor.matmul(s_ps[nb], lhsT=ones, rhs=pt[:, sl],
                                 start=(mt == 0), stop=(mt == MT - 1),
                                 skip_group_check=True)
                if mt == MT - 1:
                    nc.vector.tensor_copy(av_sb[:, 512 * nb:512 * (nb + 1)], av_ps[nb])
                    nc.vector.tensor_copy(s_sb[:, 512 * nb:512 * (nb + 1)],
                                          s_ps[nb][0:32, :])

        emit_qk(0)
        for i in range(nt):
            emit_exp(i)
            if i + 1 < nt:
                emit_qk(i + 1)
            emit_avs(i)

        # ---- tail: normalize, fin conv, bias, store ----
        with nc.allow_low_precision(reason="f32r softmax normalization"):
            nc.vector.reciprocal(r_sb, s_sb)
            for nb in range(NB):
                sl = slice(512 * nb, 512 * (nb + 1))
                rb = psum.tile([128, 512], F32, tag="av", bufs=1, name=f"rb{nb}")
                nc.tensor.matmul(rb, lhsT=inv32, rhs=r_sb[:, sl], start=True, stop=True)
                nc.vector.tensor_tensor(avn[:, sl], av_sb[:, sl], rb,
                                        mybir.AluOpType.mult)
            for oh in range(2):
                for nb in range(NB):
                    sl = slice(512 * nb, 512 * (nb + 1))
                    fp = psum.tile([128, 512], F32, tag="sp", bufs=1,
                                   name=f"fp{oh}_{nb}")
                    nc.tensor.matmul(fp, lhsT=fwt[:, 128 * oh:128 * (oh + 1)],
                                     rhs=avn[:, sl], start=True, stop=True)
                    osl = slice(NSH * oh + 512 * nb, NSH * oh + 512 * (nb + 1))
                    nc.vector.tensor_scalar_add(out_sb[:, osl], fp, tfn[:, oh:oh + 1])
                    nc.sync.dma_start(out=out_d[128 * oh:128 * (oh + 1), sl],
                                      in_=out_sb[:, osl])

      if repeat > 1:
          with tc.For_i(0, repeat, 1):
              body()
      else:
          body()

    nc.compile()
    _CACHE[key] = nc
    return nc


def _fold_bn(w, b, gamma, beta, mean, var):
    s = np.asarray(gamma, np.float32) / np.sqrt(np.asarray(var, np.float32) + BN_EPS)
    return ((np.asarray(w, np.float32) * s[:, None]).astype(np.float32),
            ((np.asarray(b, np.float32) - np.asarray(mean, np.float32)) * s
             + np.asarray(beta, np.float32)).astype(np.float32))


def _prep_in_maps(inputs):
    qdt, adt = _np_dtypes()
    thw_eff, t_th = _fold_bn(inputs["theta_w"], inputs["theta_b"], inputs["theta_gamma"],
                             inputs["theta_beta"], inputs["theta_mean"], inputs["theta_var"])
    phw_eff, t_ph = _fold_bn(inputs["phi_w"], inputs["phi_b"], inputs["phi_gamma"],
                             inputs["phi_beta"], inputs["phi_mean"], inputs["phi_var"])
    gw_eff, t_g = _fold_bn(inputs["g_w"], inputs["g_b"], inputs["g_gamma"],
                           inputs["g_beta"], inputs["g_mean"], inputs["g_var"])
    fw_eff, t_fn = _fold_bn(inputs["fin_w"], inputs["fin_b"], inputs["fin_gamma"],
                            inputs["fin_beta"], inputs["fin_mean"], inputs["fin_var"])
    t_fn_adj = (fw_eff @ t_g + t_fn).astype(np.float32)

    common = {
        "thw": np.tile(thw_eff.T, (1, 4)).astype(qdt),
        "phw": np.tile(phw_eff.T, (1, 4)).astype(qdt),
        "gwt": np.ascontiguousarray(gw_eff.T).astype(adt),
        "fwt": np.ascontiguousarray(fw_eff.T).astype(qdt),
        "ones": np.ones((128, 128), adt),
        "inv32": np.full((32, 128), 1.0 / 32.0, qdt),
        "tth": np.ascontiguousarray(np.tile(t_th, 4)[:, None]),
        "tph": np.ascontiguousarray(np.tile(t_ph, 4)[:, None]),
        "tfn": np.ascontiguousarray(t_fn_adj.reshape(2, 128).T),
    }
    xf = np.asarray(inputs["x"], np.float32).reshape(B, C, HW)
    ff = np.asarray(inputs["fea"], np.float32).reshape(B, C, HW)
    in_maps = []
    for k in range(NCORES):
        b, h = k // 2, k % 2
        m = dict(common)
        m["xk"] = np.ascontiguousarray(xf[b, :, NSH * h:NSH * (h + 1)]).astype(qdt)
        m["fea"] = np.ascontiguousarray(ff[b]).astype(qdt)
        if MM_DTYPE == "mixed":
            m["feab"] = np.ascontiguousarray(ff[b]).astype(adt)
        in_maps.append(m)
    return in_maps


def kernel(x, fea,
           theta_w, theta_b, theta_gamma, theta_beta, theta_mean, theta_var,
           phi_w, phi_b, phi_gamma, phi_beta, phi_mean, phi_var,
           g_w, g_b, g_gamma, g_beta, g_mean, g_var,
           fin_w, fin_b, fin_gamma, fin_beta, fin_mean, fin_var):
    from concourse.bass_utils import run_bass_kernel_spmd

    nc = _build()
    in_maps = _prep_in_maps(dict(
        x=x, fea=fea,
        theta_w=theta_w, theta_b=theta_b, theta_gamma=theta_gamma,
        theta_beta=theta_beta, theta_mean=theta_mean, theta_var=theta_var,
        phi_w=phi_w, phi_b=phi_b, phi_gamma=phi_gamma, phi_beta=phi_beta,
        phi_mean=phi_mean, phi_var=phi_var,
        g_w=g_w, g_b=g_b, g_gamma=g_gamma, g_beta=g_beta, g_mean=g_mean,
        g_var=g_var,
        fin_w=fin_w, fin_b=fin_b, fin_gamma=fin_gamma, fin_beta=fin_beta,
        fin_mean=fin_mean, fin_var=fin_var,
    ))
    res = run_bass_kernel_spmd(nc, in_maps, list(range(NCORES)))

    out = np.empty((B, C, HW), np.float32)
    for k in range(NCORES):
        b, h = k // 2, k % 2
        out[b, :, NSH * h:NSH * (h + 1)] = res.results[k]["out"]
    return out.reshape(B, C, 64, 64)


# revision 11
# speedup vs baseline: 1.2482x; 1.2482x over previous
"""BilinearAttention Trainium2 kernel (8 NeuronCores, SPMD, no collectives).

Problem (hardcoded): B=4, C=256, H=W=64 (HW=4096)
  theta = convbn_theta(x)   -> [B, 32, HW]
  phi   = convbn_phi(fea)   -> [B, 32, HW]
  g     = convbn_g(fea)     -> [B, 128, HW]
  attn  = softmax_m(theta^T . phi)          [B, HW(n), HW(m)]
  out   = g @ attn^T                         [B, 128, HW]
  final = convbn_fin(out)                    [B, 256, H, W]

Sharding: core k handles (b = k//2, n-half h = k%2): 2048 queries x 4096 keys.

Device algorithm per core (matmuls in f32r ~ tf32 by default, psum fp32):
  - BN folded into conv weights on host; g bias folded into fin bias
    (attn rows sum to 1); theta/phi biases applied on-chip.
  - theta_rep [128,2048]: 4 partition-group copies of theta[32, n] for
    row-tiled K=32 QK matmuls. phi_rep [128, 4096] likewise.
  - gT [128, 4096]: column block mt holds g^T of m-tile mt ([128 m, 128 c]).
  - 128 tasks (nb-major; task = (m-tile mt, n-block nb)), grouped in triples:
      QK:  logitsT[m,n] psum = phi_mt^T theta_nb  (row-packed tile_position)
      exp: ACT psum->sbuf (one [128, 3*512] instr per triple; no max
           subtraction: |logit| < ~40 so exp fits fp32 comfortably)
      AV:  av_ps[nb] += gT_mt^T . pT     (accumulate over 32 m-tiles)
      s:   s_ps[nb]  += ones^T . pT      (softmax denominators)
  - Tail: r = 1/s; broadcast via (1/32)-matmul; avn = av*r; fin conv; + bias.
"""
import os
import numpy as np
from contextlib import ExitStack

B, C, HW = 4, 256, 4096
NSH = HW // 2           # 2048 queries per core
NCORES = 8
BN_EPS = 1e-5
NB = 4                  # n-blocks of 512 per core
MT = 32                 # m-tiles of 128
TRIPLE = 3              # QK tasks per exp instruction

MM_DTYPE = os.environ.get("K_MM_DTYPE", "mixed")   # f32r | bf16 | mixed
QK_PACK = os.environ.get("K_QK_PACK", "1") == "1"

_CACHE = {}


def _np_dtypes():
    import ml_dtypes
    bf = ml_dtypes.bfloat16
    if MM_DTYPE == "bf16":
        return bf, bf
    if MM_DTYPE == "f32r":
        return np.float32, np.float32
    return np.float32, bf  # mixed: (DQ, DA)


def _build(trace_sim=False, repeat=1):
    key = ("nc", repeat, MM_DTYPE, QK_PACK)
    if key in _CACHE:
        return _CACHE[key]
    import concourse.bacc as bacc
    import concourse.tile as tile
    from concourse import mybir

    F32 = mybir.dt.float32
    # DQ: logits path (theta/phi/QK). DA: attention-weight path (pt/gT/ones/AV/s).
    DQ = mybir.dt.bfloat16 if MM_DTYPE == "bf16" else mybir.dt.float32r
    DA = mybir.dt.float32r if MM_DTYPE == "f32r" else mybir.dt.bfloat16
    AF = mybir.ActivationFunctionType

    nc = bacc.Bacc("TRN2", target_bir_lowering=False, debug=False,
                   num_devices=NCORES)

    xk_d = nc.dram_tensor("xk", [C, NSH], DQ, kind="ExternalInput").ap()
    fea_d = nc.dram_tensor("fea", [C, HW], DQ, kind="ExternalInput").ap()
    thw_d = nc.dram_tensor("thw", [C, 128], DQ, kind="ExternalInput").ap()
    phw_d = nc.dram_tensor("phw", [C, 128], DQ, kind="ExternalInput").ap()
    gwt_d = nc.dram_tensor("gwt", [C, 128], DA, kind="ExternalInput").ap()
    fwt_d = nc.dram_tensor("fwt", [128, 256], DQ, kind="ExternalInput").ap()
    ones_d = nc.dram_tensor("ones", [128, 128], DA, kind="ExternalInput").ap()
    inv32_d = nc.dram_tensor("inv32", [32, 128], DQ, kind="ExternalInput").ap()
    tth_d = nc.dram_tensor("tth", [128, 1], F32, kind="ExternalInput").ap()
    tph_d = nc.dram_tensor("tph", [128, 1], F32, kind="ExternalInput").ap()
    tfn_d = nc.dram_tensor("tfn", [128, 2], F32, kind="ExternalInput").ap()
    out_d = nc.dram_tensor("out", [256, NSH], F32, kind="ExternalOutput").ap()

    with tile.TileContext(nc, trace_sim=trace_sim) as tc, ExitStack() as ctx:
      def body():
        consts = ctx.enter_context(tc.tile_pool(name="consts", bufs=1))
        inbufs = ctx.enter_context(tc.tile_pool(name="inbufs", bufs=1))
        big = ctx.enter_context(tc.tile_pool(name="big", bufs=1))
        ptp = ctx.enter_context(tc.tile_pool(name="ptp", bufs=2))
        psum = ctx.enter_context(tc.tile_pool(name="psum", bufs=2, space="PSUM"))

        def dmac(name, shape, dt, src):
            t = consts.tile(shape, dt, name=name, tag=name)
            nc.sync.dma_start(out=t, in_=src)
            return t

        thw = [dmac(f"thw{i}", [128, 128], DQ, thw_d[128 * i:128 * (i + 1), :]) for i in range(2)]
        phw = [dmac(f"phw{i}", [128, 128], DQ, phw_d[128 * i:128 * (i + 1), :]) for i in range(2)]
        gwt = [dmac(f"gwt{i}", [128, 128], DA, gwt_d[128 * i:128 * (i + 1), :]) for i in range(2)]
        fwt = dmac("fwt_t", [128, 256], DQ, fwt_d)
        ones = dmac("ones_t", [128, 128], DA, ones_d)
        inv32 = dmac("inv32_t", [32, 128], DQ, inv32_d)
        tth = dmac("tth_t", [128, 1], F32, tth_d)
        tph = dmac("tph_t", [128, 1], F32, tph_d)
        tfn = dmac("tfn_t", [128, 2], F32, tfn_d)

        theta_rep = [big.tile([128, 512], DQ, name=f"theta_rep{j}", tag=f"theta_rep{j}")
                     for j in range(4)]
        phi_rep = [big.tile([128, 512], DQ, name=f"phi_rep{j}", tag=f"phi_rep{j}")
                   for j in range(8)]
        gT = [big.tile([128, 512], DA, name=f"gT{j}", tag=f"gT{j}") for j in range(8)]
        av_sb = big.tile([128, NSH], F32)
        avn = big.tile([128, NSH], DQ)
        s_sb = big.tile([32, NSH], DQ)
        r_sb = big.tile([32, NSH], DQ)
        out_sb = big.tile([128, 2 * NSH], F32)

        # ---- prologue: bulk input DMAs (few, large) ----
        xk0 = inbufs.tile([128, NSH], DQ, tag="xk0")
        xk1 = inbufs.tile([128, NSH], DQ, tag="xk1")
        nc.sync.dma_start(out=xk0, in_=xk_d[0:128, :])
        nc.sync.dma_start(out=xk1, in_=xk_d[128:256, :])
        f0 = inbufs.tile([128, HW], DQ, tag="f0")
        f1 = inbufs.tile([128, HW], DQ, tag="f1")
        nc.sync.dma_start(out=f0, in_=fea_d[0:128, :])
        nc.sync.dma_start(out=f1, in_=fea_d[128:256, :])
        if MM_DTYPE == "mixed":
            f0b = inbufs.tile([128, HW], DA, tag="f0b")
            f1b = inbufs.tile([128, HW], DA, tag="f1b")
            nc.vector.tensor_copy(f0b, f0)
            nc.vector.tensor_copy(f1b, f1)
        else:
            f0b, f1b = f0, f1

        # ---- prologue: theta conv over the core's n-range ----
        for j in range(4):
            sl = slice(512 * j, 512 * (j + 1))
            ps = psum.tile([128, 512], F32, tag="qk")
            nc.tensor.matmul(ps, lhsT=thw[0], rhs=xk0[:, sl], start=True, stop=False)
            nc.tensor.matmul(ps, lhsT=thw[1], rhs=xk1[:, sl], start=False, stop=True)
            nc.vector.tensor_scalar_add(theta_rep[j], ps, tth)

        # ---- prologue: phi conv + gT conv per 512-wide m-chunk ----
        for j in range(8):
            sl = slice(512 * j, 512 * (j + 1))
            ps = psum.tile([128, 512], F32, tag="qk")
            nc.tensor.matmul(ps, lhsT=phw[0], rhs=f0[:, sl], start=True, stop=False)
            nc.tensor.matmul(ps, lhsT=phw[1], rhs=f1[:, sl], start=False, stop=True)
            nc.vector.tensor_scalar_add(phi_rep[j], ps, tph)
            ps2 = psum.tile([128, 512], F32, tag="qk")
            for t in range(4):
                slc = slice(128 * t, 128 * (t + 1))
                gsl = slice(512 * j + 128 * t, 512 * j + 128 * (t + 1))
                nc.tensor.matmul(ps2[:, slc], lhsT=f0b[:, gsl], rhs=gwt[0],
                                 start=True, stop=False)
                nc.tensor.matmul(ps2[:, slc], lhsT=f1b[:, gsl], rhs=gwt[1],
                                 start=False, stop=True)
            nc.vector.tensor_copy(gT[j], ps2)

        # ---- main loop ----
        tasks = [(i % MT, i // MT) for i in range(MT * NB)]  # (mt, nb), nb-major
        triples = [tasks[i:i + TRIPLE] for i in range(0, len(tasks), TRIPLE)]
        nt = len(triples)
        av_ps = [None] * NB
        s_ps = [None] * NB
        quads = [None] * nt
        pts = [None] * nt

        def emit_qk(i):
            grp = triples[i]
            q = psum.tile([128, 512 * len(grp)], F32, tag="qk", name=f"q{i}")
            quads[i] = q
            for jj, (mt, nb) in enumerate(grp):
                r = (mt % 4) if QK_PACK else 0
                kw = dict(tile_position=(32 * r, 0)) if QK_PACK else {}
                pc = phi_rep[mt // 4]
                mo = 128 * (mt % 4)
                nc.tensor.matmul(
                    q[:, 512 * jj:512 * (jj + 1)],
                    lhsT=pc[32 * r:32 * (r + 1), mo:mo + 128],
                    rhs=theta_rep[nb][32 * r:32 * (r + 1), :],
                    start=True, stop=True, **kw,
                )

        def emit_exp(i):
            q = quads[i]
            pt = ptp.tile([128, q.shape[-1]], DA, tag="pt", name=f"pt{i}")
            pts[i] = pt
            nc.scalar.activation(out=pt, in_=q, func=AF.Exp)

        def emit_avs(i):
            grp = triples[i]
            pt = pts[i]
            for jj, (mt, nb) in enumerate(grp):
                if mt == 0:
                    av_ps[nb] = psum.tile([128, 512], F32, tag="av", bufs=1,
                                          name=f"av_ps{nb}")
                    s_ps[nb] = psum.tile([128, 512], F32, tag="sp", bufs=1,
                                         name=f"s_ps{nb}")
                sl = slice(512 * jj, 512 * (jj + 1))
                gc = gT[mt // 4]
                go = 128 * (mt % 4)
                nc.tensor.matmul(av_ps[nb], lhsT=gc[:, go:go + 128],
                                 rhs=pt[:, sl], start=(mt == 0), stop=(mt == MT - 1),
                                 skip_group_check=True)
                nc.tensor.matmul(s_ps[nb], lhsT=ones, rhs=pt[:, sl],
                                 start=(mt == 0), stop=(mt == MT - 1),
                                 skip_group_check=True)
                if mt == MT - 1:
                    psl = slice(512 * nb, 512 * (nb + 1))
                    nc.vector.tensor_copy(av_sb[:, psl], av_ps[nb])
                    nc.vector.tensor_copy(s_sb[:, psl], s_ps[nb][0:32, :])
                    emit_pass_tail(nb)

        def emit_pass_tail(nb):
            sl = slice(512 * nb, 512 * (nb + 1))
            with nc.allow_low_precision(reason="f32r softmax normalization"):
                nc.vector.reciprocal(r_sb[:, sl], s_sb[:, sl])
                rb = psum.tile([128, 512], F32, tag="qk", name=f"rb{nb}")
                nc.tensor.matmul(rb, lhsT=inv32, rhs=r_sb[:, sl], start=True, stop=True)
                nc.vector.tensor_tensor(avn[:, sl], av_sb[:, sl], rb,
                                        mybir.AluOpType.mult)
                for oh in range(2):
                    fp = psum.tile([128, 512], F32, tag="qk", name=f"fp{oh}_{nb}")
                    nc.tensor.matmul(fp, lhsT=fwt[:, 128 * oh:128 * (oh + 1)],
                                     rhs=avn[:, sl], start=True, stop=True)
                    osl = slice(NSH * oh + 512 * nb, NSH * oh + 512 * (nb + 1))
                    nc.vector.tensor_scalar_add(out_sb[:, osl], fp, tfn[:, oh:oh + 1])
                    nc.scalar.dma_start(out=out_d[128 * oh:128 * (oh + 1), sl],
                                        in_=out_sb[:, osl])

        emit_qk(0)
        for i in range(nt):
            emit_exp(i)
            if i + 1 < nt:
                emit_qk(i + 1)
            emit_avs(i)

      if repeat > 1:
          with tc.For_i(0, repeat, 1):
              body()
      else:
          body()

    nc.compile()
    _CACHE[key] = nc
    return nc


def _fold_bn(w, b, gamma, beta, mean, var):
    s = np.asarray(gamma, np.float32) / np.sqrt(np.asarray(var, np.float32) + BN_EPS)
    return ((np.asarray(w, np.float32) * s[:, None]).astype(np.float32),
            ((np.asarray(b, np.float32) - np.asarray(mean, np.float32)) * s
             + np.asarray(beta, np.float32)).astype(np.float32))


def _prep_in_maps(inputs):
    qdt, adt = _np_dtypes()
    thw_eff, t_th = _fold_bn(inputs["theta_w"], inputs["theta_b"], inputs["theta_gamma"],
                             inputs["theta_beta"], inputs["theta_mean"], inputs["theta_var"])
    phw_eff, t_ph = _fold_bn(inputs["phi_w"], inputs["phi_b"], inputs["phi_gamma"],
                             inputs["phi_beta"], inputs["phi_mean"], inputs["phi_var"])
    gw_eff, t_g = _fold_bn(inputs["g_w"], inputs["g_b"], inputs["g_gamma"],
                           inputs["g_beta"], inputs["g_mean"], inputs["g_var"])
    fw_eff, t_fn = _fold_bn(inputs["fin_w"], inputs["fin_b"], inputs["fin_gamma"],
                            inputs["fin_beta"], inputs["fin_mean"], inputs["fin_var"])
    t_fn_adj = (fw_eff @ t_g + t_fn).astype(np.float32)

    common = {
        "thw": np.tile(thw_eff.T, (1, 4)).astype(qdt),
        "phw": np.tile(phw_eff.T, (1, 4)).astype(qdt),
        "gwt": np.ascontiguousarray(gw_eff.T).astype(adt),
        "fwt": np.ascontiguousarray(fw_eff.T).astype(qdt),
        "ones": np.ones((128, 128), adt),
        "inv32": np.full((32, 128), 1.0 / 32.0, qdt),
        "tth": np.ascontiguousarray(np.tile(t_th, 4)[:, None]),
        "tph": np.ascontiguousarray(np.tile(t_ph, 4)[:, None]),
        "tfn": np.ascontiguousarray(t_fn_adj.reshape(2, 128).T),
    }
    xf = np.asarray(inputs["x"], np.float32).reshape(B, C, HW)
    ff = np.asarray(inputs["fea"], np.float32).reshape(B, C, HW)
    in_maps = []
    for k in range(NCORES):
        b, h = k // 2, k % 2
        m = dict(common)
        m["xk"] = np.ascontiguousarray(xf[b, :, NSH * h:NSH * (h + 1)]).astype(qdt)
        m["fea"] = np.ascontiguousarray(ff[b]).astype(qdt)
        in_maps.append(m)
    return in_maps


def kernel(x, fea,
           theta_w, theta_b, theta_gamma, theta_beta, theta_mean, theta_var,
           phi_w, phi_b, phi_gamma, phi_beta, phi_mean, phi_var,
           g_w, g_b, g_gamma, g_beta, g_mean, g_var,
           fin_w, fin_b, fin_gamma, fin_beta, fin_mean, fin_var):
    from concourse.bass_utils import run_bass_kernel_spmd

    nc = _build()
    in_maps = _prep_in_maps(dict(
        x=x, fea=fea,
        theta_w=theta_w, theta_b=theta_b, theta_gamma=theta_gamma,
        theta_beta=theta_beta, theta_mean=theta_mean, theta_var=theta_var,
        phi_w=phi_w, phi_b=phi_b, phi_gamma=phi_gamma, phi_beta=phi_beta,
        phi_mean=phi_mean, phi_var=phi_var,
        g_w=g_w, g_b=g_b, g_gamma=g_gamma, g_beta=g_beta, g_mean=g_mean,
        g_var=g_var,
        fin_w=fin_w, fin_b=fin_b, fin_gamma=fin_gamma, fin_beta=fin_beta,
        fin_mean=fin_mean, fin_var=fin_var,
    ))
    res = run_bass_kernel_spmd(nc, in_maps, list(range(NCORES)))

    out = np.empty((B, C, HW), np.float32)
    for k in range(NCORES):
        b, h = k // 2, k % 2
        out[b, :, NSH * h:NSH * (h + 1)] = res.results[k]["out"]
    return out.reshape(B, C, 64, 64)


# revision 16
# speedup vs baseline: 16145.4872x; 12934.7630x over previous
"""BilinearAttention Trainium2 kernel (8 NeuronCores, SPMD, no collectives).

Problem (hardcoded): B=4, C=256, H=W=64 (HW=4096)
  theta = convbn_theta(x)   -> [B, 32, HW]
  phi   = convbn_phi(fea)   -> [B, 32, HW]
  g     = convbn_g(fea)     -> [B, 128, HW]
  attn  = softmax_m(theta^T . phi)          [B, HW(n), HW(m)]
  out   = g @ attn^T                         [B, 128, HW]
  final = convbn_fin(out)                    [B, 256, H, W]

Sharding: core k handles (b = k//2, n-half h = k%2): 2048 queries x 4096 keys.

Device algorithm per core (matmuls in f32r ~ tf32 by default, psum fp32):
  - BN folded into conv weights on host; g bias folded into fin bias
    (attn rows sum to 1); theta/phi biases applied on-chip.
  - theta_rep [128,2048]: 4 partition-group copies of theta[32, n] for
    row-tiled K=32 QK matmuls. phi_rep [128, 4096] likewise.
  - gT [128, 4096]: column block mt holds g^T of m-tile mt ([128 m, 128 c]).
  - 128 tasks (nb-major; task = (m-tile mt, n-block nb)), grouped in triples:
      QK:  logitsT[m,n] psum = phi_mt^T theta_nb  (row-packed tile_position)
      exp: ACT psum->sbuf (one [128, 3*512] instr per triple; no max
           subtraction: |logit| < ~40 so exp fits fp32 comfortably)
      AV:  av_ps[nb] += gT_mt^T . pT     (accumulate over 32 m-tiles)
      s:   s_ps[nb]  += ones^T . pT      (softmax denominators)
  - Tail: r = 1/s; broadcast via (1/32)-matmul; avn = av*r; fin conv; + bias.
"""
import os
import numpy as np
from contextlib import ExitStack

B, C, HW = 4, 256, 4096
NSH = HW // 2           # 2048 queries per core
NCORES = 8
BN_EPS = 1e-5
NB = 4                  # n-blocks of 512 per core
MT = 32                 # m-tiles of 128
TRIPLE = 3              # QK tasks per exp instruction

MM_DTYPE = os.environ.get("K_MM_DTYPE", "mixed")   # f32r | bf16 | mixed
QK_PACK = os.environ.get("K_QK_PACK", "1") == "1"

_CACHE = {}


def _np_dtypes():
    import ml_dtypes
    bf = ml_dtypes.bfloat16
    if MM_DTYPE == "bf16":
        return bf, bf
    if MM_DTYPE == "f32r":
        return np.float32, np.float32
    return np.float32, bf  # mixed: (DQ, DA)


def _build(trace_sim=False, repeat=1):
    key = ("nc", repeat, MM_DTYPE, QK_PACK)
    if key in _CACHE:
        return _CACHE[key]
    import concourse.bacc as bacc
    import concourse.tile as tile
    from concourse import mybir

    F32 = mybir.dt.float32
    # DQ: logits path (theta/phi/QK). DA: attention-weight path (pt/gT/ones/AV/s).
    DQ = mybir.dt.bfloat16 if MM_DTYPE == "bf16" else mybir.dt.float32r
    DA = mybir.dt.float32r if MM_DTYPE == "f32r" else mybir.dt.bfloat16
    AF = mybir.ActivationFunctionType

    nc = bacc.Bacc("TRN2", target_bir_lowering=False, debug=False,
                   num_devices=NCORES)

    xk_d = nc.dram_tensor("xk", [C, NSH], DQ, kind="ExternalInput").ap()
    fea_d = nc.dram_tensor("fea", [C, HW], DQ, kind="ExternalInput").ap()
    thw_d = nc.dram_tensor("thw", [C, 128], DQ, kind="ExternalInput").ap()
    phw_d = nc.dram_tensor("phw", [C, 128], DQ, kind="ExternalInput").ap()
    gwt_d = nc.dram_tensor("gwt", [C, 128], DA, kind="ExternalInput").ap()
    fwt_d = nc.dram_tensor("fwt", [128, 256], DQ, kind="ExternalInput").ap()
    ones_d = nc.dram_tensor("ones", [128, 128], DA, kind="ExternalInput").ap()
    inv32_d = nc.dram_tensor("inv32", [32, 128], DQ, kind="ExternalInput").ap()
    tth_d = nc.dram_tensor("tth", [128, 1], F32, kind="ExternalInput").ap()
    tph_d = nc.dram_tensor("tph", [128, 1], F32, kind="ExternalInput").ap()
    tfn_d = nc.dram_tensor("tfn", [128, 2], F32, kind="ExternalInput").ap()
    out_d = nc.dram_tensor("out", [256, NSH], F32, kind="ExternalOutput").ap()

    with tile.TileContext(nc, trace_sim=trace_sim) as tc, ExitStack() as ctx:
      def body():
        consts = ctx.enter_context(tc.tile_pool(name="consts", bufs=1))
        inbufs = ctx.enter_context(tc.tile_pool(name="inbufs", bufs=1))
        big = ctx.enter_context(tc.tile_pool(name="big", bufs=1))
        ptp = ctx.enter_context(tc.tile_pool(name="ptp", bufs=2))
        psum = ctx.enter_context(tc.tile_pool(name="psum", bufs=2, space="PSUM"))

        def dmac(name, shape, dt, src):
            t = consts.tile(shape, dt, name=name, tag=name)
            nc.sync.dma_start(out=t, in_=src)
            return t

        thw = [dmac(f"thw{i}", [128, 128], DQ, thw_d[128 * i:128 * (i + 1), :]) for i in range(2)]
        phw = [dmac(f"phw{i}", [128, 128], DQ, phw_d[128 * i:128 * (i + 1), :]) for i in range(2)]
        gwt = [dmac(f"gwt{i}", [128, 128], DA, gwt_d[128 * i:128 * (i + 1), :]) for i in range(2)]
        fwt = dmac("fwt_t", [128, 256], DQ, fwt_d)
        ones = dmac("ones_t", [128, 128], DA, ones_d)
        inv32 = dmac("inv32_t", [32, 128], DQ, inv32_d)
        tth = dmac("tth_t", [128, 1], F32, tth_d)
        tph = dmac("tph_t", [128, 1], F32, tph_d)
        tfn = dmac("tfn_t", [128, 2], F32, tfn_d)

        theta_rep = [big.tile([128, 512], DQ, name=f"theta_rep{j}", tag=f"theta_rep{j}")
                     for j in range(4)]
        phi_rep = [big.tile([128, 512], DQ, name=f"phi_rep{j}", tag=f"phi_rep{j}")
                   for j in range(8)]
        gT = [big.tile([128, 512], DA, name=f"gT{j}", tag=f"gT{j}") for j in range(8)]
        av_sb = big.tile([128, NSH], F32)
        avn = big.tile([128, NSH], DQ)
        s_sb = big.tile([32, NSH], DQ)
        r_sb = big.tile([32, NSH], DQ)
        out_sb = big.tile([128, 2 * NSH], F32)

        # ---- prologue: bulk input DMAs (few, large) ----
        xk0 = inbufs.tile([128, NSH], DQ, tag="xk0")
        xk1 = inbufs.tile([128, NSH], DQ, tag="xk1")
        nc.sync.dma_start(out=xk0[:, 0:1024], in_=xk_d[0:128, 0:1024])
        nc.scalar.dma_start(out=xk1[:, 0:1024], in_=xk_d[128:256, 0:1024])
        nc.sync.dma_start(out=xk0[:, 1024:2048], in_=xk_d[0:128, 1024:2048])
        nc.scalar.dma_start(out=xk1[:, 1024:2048], in_=xk_d[128:256, 1024:2048])
        f0 = inbufs.tile([128, HW], DQ, tag="f0")
        f1 = inbufs.tile([128, HW], DQ, tag="f1")
        nc.sync.dma_start(out=f0, in_=fea_d[0:128, :])
        nc.scalar.dma_start(out=f1, in_=fea_d[128:256, :])
        if MM_DTYPE == "mixed":
            f0b = inbufs.tile([128, HW], DA, tag="f0b")
            f1b = inbufs.tile([128, HW], DA, tag="f1b")
        else:
            f0b, f1b = f0, f1

        # ---- prologue: theta conv over the core's n-range ----
        for j in range(4):
            sl = slice(512 * j, 512 * (j + 1))
            ps = psum.tile([128, 512], F32, tag="qk")
            nc.tensor.matmul(ps, lhsT=thw[0], rhs=xk0[:, sl], start=True, stop=False)
            nc.tensor.matmul(ps, lhsT=thw[1], rhs=xk1[:, sl], start=False, stop=True)
            nc.vector.tensor_scalar_add(theta_rep[j], ps, tth)

        # ---- phi conv + gT conv per 512-wide m-chunk (emitted just-in-time) ----
        conv_done = set()

        def emit_conv_chunk(j):
            if j in conv_done or j >= 8:
                return
            conv_done.add(j)
            sl = slice(512 * j, 512 * (j + 1))
            if MM_DTYPE == "mixed":
                nc.vector.tensor_copy(f0b[:, sl], f0[:, sl])
                nc.vector.tensor_copy(f1b[:, sl], f1[:, sl])
            ps = psum.tile([128, 512], F32, tag="qk", name=f"phps{j}")
            nc.tensor.matmul(ps, lhsT=phw[0], rhs=f0[:, sl], start=True, stop=False)
            nc.tensor.matmul(ps, lhsT=phw[1], rhs=f1[:, sl], start=False, stop=True)
            nc.vector.tensor_scalar_add(phi_rep[j], ps, tph)
            ps2 = psum.tile([128, 512], F32, tag="qk", name=f"gtps{j}")
            for t in range(4):
                slc = slice(128 * t, 128 * (t + 1))
                gsl = slice(512 * j + 128 * t, 512 * j + 128 * (t + 1))
                nc.tensor.matmul(ps2[:, slc], lhsT=f0b[:, gsl], rhs=gwt[0],
                                 start=True, stop=False)
                nc.tensor.matmul(ps2[:, slc], lhsT=f1b[:, gsl], rhs=gwt[1],
                                 start=False, stop=True)
            nc.vector.tensor_copy(gT[j], ps2)

        # ---- main loop ----
        tasks = [(i % MT, i // MT) for i in range(MT * NB)]  # (mt, nb), nb-major
        triples = [tasks[i:i + TRIPLE] for i in range(0, len(tasks), TRIPLE)]
        nt = len(triples)
        av_ps = [None] * NB
        s_ps = [None] * NB
        quads = [None] * nt
        pts = [None] * nt

        def emit_qk(i):
            grp = triples[i]
            q = psum.tile([128, 512 * len(grp)], F32, tag="qk", name=f"q{i}")
            quads[i] = q
            for jj, (mt, nb) in enumerate(grp):
                r = (mt % 4) if QK_PACK else 0
                kw = dict(tile_position=(32 * r, 0)) if QK_PACK else {}
                pc = phi_rep[mt // 4]
                mo = 128 * (mt % 4)
                nc.tensor.matmul(
                    q[:, 512 * jj:512 * (jj + 1)],
                    lhsT=pc[32 * r:32 * (r + 1), mo:mo + 128],
                    rhs=theta_rep[nb][32 * r:32 * (r + 1), :],
                    start=True, stop=True, **kw,
                )

        def emit_exp(i):
            q = quads[i]
            pt = ptp.tile([128, q.shape[-1]], DA, tag="pt", name=f"pt{i}")
            pts[i] = pt
            nc.scalar.activation(out=pt, in_=q, func=AF.Exp)

        def emit_avs(i):
            grp = triples[i]
            pt = pts[i]
            for jj, (mt, nb) in enumerate(grp):
                if mt == 0:
                    av_ps[nb] = psum.tile([128, 512], F32, tag="av", bufs=1,
                                          name=f"av_ps{nb}")
                    s_ps[nb] = psum.tile([128, 512], F32, tag="sp", bufs=1,
                                         name=f"s_ps{nb}")
                sl = slice(512 * jj, 512 * (jj + 1))
                gc = gT[mt // 4]
                go = 128 * (mt % 4)
                nc.tensor.matmul(av_ps[nb], lhsT=gc[:, go:go + 128],
                                 rhs=pt[:, sl], start=(mt == 0), stop=(mt == MT - 1),
                                 skip_group_check=True)
                nc.tensor.matmul(s_ps[nb], lhsT=ones, rhs=pt[:, sl],
                                 start=(mt == 0), stop=(mt == MT - 1),
                                 skip_group_check=True)
                if mt == MT - 1:
                    psl = slice(512 * nb, 512 * (nb + 1))
                    nc.vector.tensor_copy(av_sb[:, psl], av_ps[nb])
                    nc.vector.tensor_copy(s_sb[:, psl], s_ps[nb][0:32, :])
                    pending_tails.append(nb)

        def emit_pass_tail(nb):
            sl = slice(512 * nb, 512 * (nb + 1))
            with nc.allow_low_precision(reason="f32r softmax normalization"):
                nc.vector.reciprocal(r_sb[:, sl], s_sb[:, sl])
                rb = psum.tile([128, 512], F32, tag="qk", name=f"rb{nb}")
                nc.tensor.matmul(rb, lhsT=inv32, rhs=r_sb[:, sl], start=True, stop=True)
                nc.vector.tensor_tensor(avn[:, sl], av_sb[:, sl], rb,
                                        mybir.AluOpType.mult)
                for oh in range(2):
                    fp = psum.tile([128, 512], F32, tag="qk", name=f"fp{oh}_{nb}")
                    nc.tensor.matmul(fp, lhsT=fwt[:, 128 * oh:128 * (oh + 1)],
                                     rhs=avn[:, sl], start=True, stop=True)
                    osl = slice(NSH * oh + 512 * nb, NSH * oh + 512 * (nb + 1))
                    nc.vector.tensor_scalar_add(out_sb[:, osl], fp, tfn[:, oh:oh + 1])
                    nc.sync.dma_start(out=out_d[128 * oh:128 * (oh + 1), sl],
                                      in_=out_sb[:, osl])

        pending_tails = []
        tail_delay = {}
        emit_conv_chunk(0)
        emit_qk(0)
        for i in range(nt):
            emit_exp(i)
            if i + 1 < nt:
                for mt, _nb in triples[i + 1]:
                    emit_conv_chunk(mt // 4)
                emit_qk(i + 1)
            emit_avs(i)
            for nb in list(pending_tails):
                tail_delay[nb] = tail_delay.get(nb, 0) + 1
                if tail_delay[nb] >= 2 or i == nt - 1:
                    pending_tails.remove(nb)
                    emit_pass_tail(nb)
        for nb in pending_tails:
            emit_pass_tail(nb)

      if repeat > 1:
          from concourse import mybir as _mb
          with tc.For_i(0, repeat, 1, hint_engines=(
                  _mb.EngineType.PE, _mb.EngineType.Activation,
                  _mb.EngineType.DVE, _mb.EngineType.SP, _mb.EngineType.Pool)):
              body()
      else:
          body()

    nc.compile()
    _CACHE[key] = nc
    return nc


def _fold_bn(w, b, gamma, beta, mean, var):
    s = np.asarray(gamma, np.float32) / np.sqrt(np.asarray(var, np.float32) + BN_EPS)
    return ((np.asarray(w, np.float32) * s[:, None]).astype(np.float32),
            ((np.asarray(b, np.float32) - np.asarray(mean, np.float32)) * s
             + np.asarray(beta, np.float32)).astype(np.float32))


def _prep_in_maps(inputs):
    qdt, adt = _np_dtypes()
    thw_eff, t_th = _fold_bn(inputs["theta_w"], inputs["theta_b"], inputs["theta_gamma"],
                             inputs["theta_beta"], inputs["theta_mean"], inputs["theta_var"])
    phw_eff, t_ph = _fold_bn(inputs["phi_w"], inputs["phi_b"], inputs["phi_gamma"],
                             inputs["phi_beta"], inputs["phi_mean"], inputs["phi_var"])
    gw_eff, t_g = _fold_bn(inputs["g_w"], inputs["g_b"], inputs["g_gamma"],
                           inputs["g_beta"], inputs["g_mean"], inputs["g_var"])
    fw_eff, t_fn = _fold_bn(inputs["fin_w"], inputs["fin_b"], inputs["fin_gamma"],
                            inputs["fin_beta"], inputs["fin_mean"], inputs["fin_var"])
    t_fn_adj = (fw_eff @ t_g + t_fn).astype(np.float32)

    common = {
        "thw": np.tile(thw_eff.T, (1, 4)).astype(qdt),
        "phw": np.tile(phw_eff.T, (1, 4)).astype(qdt),
        "gwt": np.ascontiguousarray(gw_eff.T).astype(adt),
        "fwt": np.ascontiguousarray(fw_eff.T).astype(qdt),
        "ones": np.ones((128, 128), adt),
        "inv32": np.full((32, 128), 1.0 / 32.0, qdt),
        "tth": np.ascontiguousarray(np.tile(t_th, 4)[:, None]),
        "tph": np.ascontiguousarray(np.tile(t_ph, 4)[:, None]),
        "tfn": np.ascontiguousarray(t_fn_adj.reshape(2, 128).T),
    }
    xf = np.asarray(inputs["x"], np.float32).reshape(B, C, HW)
    ff = np.asarray(inputs["fea"], np.float32).reshape(B, C, HW)
    in_maps = []
    for k in range(NCORES):
        b, h = k // 2, k % 2
        m = dict(common)
        m["xk"] = np.ascontiguousarray(xf[b, :, NSH * h:NSH * (h + 1)]).astype(qdt)
        m["fea"] = np.ascontiguousarray(ff[b]).astype(qdt)
        in_maps.append(m)
    return in_maps


def kernel(x, fea,
           theta_w, theta_b, theta_gamma, theta_beta, theta_mean, theta_var,
           phi_w, phi_b, phi_gamma, phi_beta, phi_mean, phi_var,
           g_w, g_b, g_gamma, g_beta, g_mean, g_var,
           fin_w, fin_b, fin_gamma, fin_beta, fin_mean, fin_var):
    from concourse.bass_utils import run_bass_kernel_spmd

    nc = _build()
    in_maps = _prep_in_maps(dict(
        x=x, fea=fea,
        theta_w=theta_w, theta_b=theta_b, theta_gamma=theta_gamma,
        theta_beta=theta_beta, theta_mean=theta_mean, theta_var=theta_var,
        phi_w=phi_w, phi_b=phi_b, phi_gamma=phi_gamma, phi_beta=phi_beta,
        phi_mean=phi_mean, phi_var=phi_var,
        g_w=g_w, g_b=g_b, g_gamma=g_gamma, g_beta=g_beta, g_mean=g_mean,
        g_var=g_var,
        fin_w=fin_w, fin_b=fin_b, fin_gamma=fin_gamma, fin_beta=fin_beta,
        fin_mean=fin_mean, fin_var=fin_var,
    ))
    res = run_bass_kernel_spmd(nc, in_maps, list(range(NCORES)))

    out = np.empty((B, C, HW), np.float32)
    for k in range(NCORES):
        b, h = k // 2, k % 2
        out[b, :, NSH * h:NSH * (h + 1)] = res.results[k]["out"]
    return out.reshape(B, C, 64, 64)
